# revision 23
# baseline (speedup 1.0000x reference)
"""Trainium2 Bass kernel for nn_EqualityConstrainedQuadratic.

Mathematical structure (verified against the reference):
  - The per-sample KKT matrices are identical across the batch (Hessian of
    f is M, jacrev(F) wrt x is A0, b = -F(0,0) = -c), so the batch shares one
    576x576 saddle solve with per-sample right-hand sides; B0 never matters.
  - With H = M/2 + I and Mt = M + 2I (spectrum ~[2, 7]):
        y = Y1 - U Si (A0 Y1 + c),  [Y1|U] = Z,  Mt Z = 2 [x-parms | A0^T],
        S = A0 U, Si = S^-1.
  - Single bf16 heavy-ball solve (K=7 iterations, constant coefficients
    alpha/beta), no iterative refinement: the bf16(M) quantization floor is
    ~3-4e-3 relative, well under the 2e-2 gate.

Execution design (per core; data parallel over batch, 16 samples/core):
  - Heavy-ball in "increment" form: d_{k+1} = beta d_k + alpha r_k,
    x_{k+1} = x_k + d_{k+1}, r_{k+1} = r_k - Mt d_{k+1}.  The scaled
    residual R = alpha*r lives IN PSUM as a running matmul accumulation
    R += (-alpha Mt) @ d_k with the matrix scale folded on the host, so the
    whole per-sweep update is ONE single-scalar DVE STT per half:
    d' = beta*d + R (in1 read straight from PSUM).  No ACT in the loop.
  - The solution x = sum_k d_k accumulates in a second PSUM bank via an
    identity matmul.  The first sweep uses modified diagonal blocks
    bf16(I - alpha Mt), which folds the R init in (alpha r_0 = d_1).
  - R is split into two PSUM tiles (halves) so the DVE updates don't wait
    on the full 16-matmul sweep (Tile tracks deps per tile).
  - PSUM accumulation gotcha: start_tensor_calc marks the WHOLE 2KB bank
    pending-zero, so each bank gets start=True exactly once.
  - The Schur inverse runs via Newton-Schulz on an EARLY approximate S
    (from the partially converged U) with a quadratic polynomial init,
    overlapped with the remaining sweeps; one fp32 NS polish against the
    final S in the tail.
  - Output is written in column layout and transposed on the host;
    host pre-marshals layouts/dtypes only (transposed tiles, bf16 casts of
    scaled Mt, identity); all input-data math runs on device.
  - PE is kept warm through the input DMA with dummy matmuls (HAM ramps to
    2.4 GHz after ~3.4us of activity; all DMA on the two HWDGE rings since
    gpsimd/SWDGE boots ~6us late).
"""

import os
import sys

import numpy as np

for _p in ("/root/.axon_site", "/root/.axon_site/_ro/trn_rl_repo"):
    if os.path.isdir(_p) and _p not in sys.path:
        sys.path.append(_p)

import ml_dtypes

import concourse.mybir as mybir
from concourse import bacc
from concourse.bass_utils import run_bass_kernel_spmd
from concourse.tile import TileContext

F32 = mybir.dt.float32
BF16 = mybir.dt.bfloat16
OP = mybir.AluOpType
AF = mybir.ActivationFunctionType
BF = ml_dtypes.bfloat16

# problem shape (hardcoded per contract)
B, N, E = 128, 512, 64
NCORES = 8
BS = B // NCORES  # 16 samples per core
NB = N // 128  # 4 row blocks
W = BS + E  # 80 state columns per core
TW = NB * W  # 320

# spectral bounds of Mt = M + 2I -> heavy-ball coefficients
A_LO, A_HI = 2.0, 7.5
ALPHA = (2.0 / (A_HI**0.5 + A_LO**0.5)) ** 2
BETA = ((A_HI / A_LO) ** 0.5 - 1.0) ** 2 / ((A_HI / A_LO) ** 0.5 + 1.0) ** 2

K = int(os.environ.get("KERNEL_K", "6"))  # heavy-ball iterations (K-1 sweeps)
S_EARLY = max(1, K - 4)  # build approximate S from the partial solution
J_E = 2  # NS bf16 iterations on early S
J_F = 1  # NS fp32 polish iterations on final S
A_NS, B_NS = 2.5, -1.023  # NS quadratic init X0 = a I + b S
DUMMY_N = int(os.environ.get("KERNEL_DUMMY", "42"))  # PE warm-up matmuls

S0 = 2.0 * ALPHA  # d_1 = alpha * 2 * [q | A0^T]

# bf16 blob column layout: [a0tb | idb | mtg | mtb]
C_A0 = 0
C_ID = C_A0 + NB * E  # 256
C_MG = C_ID + 128  # 384
C_MT = C_MG + NB * 128  # 896
C16 = C_MT + NB * N  # 2944
# f32 blob column layout: [xt | pt | aeye | cpos]
F_XT = 0
F_PT = F_XT + NB * BS  # 64
F_AE = F_PT + NB * BS  # 128
F_CP = F_AE + E  # 192
C32 = F_CP + BS  # 208

LAST_RUN = {}


def build_bass(dbg=False):
    nc = bacc.Bacc("TRN2", target_bir_lowering=False)

    b16_d = nc.dram_tensor("b16", [128, C16], BF16, kind="ExternalInput")
    b32_d = nc.dram_tensor("b32", [128, C32], F32, kind="ExternalInput")
    # y in column layout [128, m*BS+j]; the host transposes back
    y_d = nc.dram_tensor("y", [128, NB * BS], F32, kind="ExternalOutput")
    if dbg:
        z_dbg = nc.dram_tensor("dbg_z", [128, TW], F32, kind="ExternalOutput")
        s_dbg = nc.dram_tensor("dbg_s", [E, E], F32, kind="ExternalOutput")
        x_dbg = nc.dram_tensor("dbg_x", [E, E], F32, kind="ExternalOutput")
        w_dbg = nc.dram_tensor("dbg_w", [E, BS], F32, kind="ExternalOutput")

    with TileContext(nc) as tc:
        with (
            tc.tile_pool(name="consts", bufs=1) as consts,
            tc.tile_pool(name="state", bufs=1) as state,
            tc.tile_pool(name="pz", bufs=1, space="PSUM") as pz,
        ):
            b16 = consts.tile([128, C16], BF16, tag="b16")
            b32 = consts.tile([128, C32], F32, tag="b32")
            a0tb = b16[:, C_A0:C_ID]
            idb = b16[:, C_ID:C_MG]
            mtg = b16[:, C_MG:C_MT]
            mtb = b16[:, C_MT:C16]
            xt = b32[:, F_XT:F_PT]
            pt = b32[:, F_PT:F_AE]
            aeye = b32[0:E, F_AE:F_CP]
            cpos = b32[0:E, F_CP:C32]

            warm = consts.tile([128, 128], BF16, tag="warm")
            q = state.tile([128, NB * BS], F32, tag="q")
            ga = state.tile([128, TW], BF16, tag="ga")
            gb = state.tile([128, TW], BF16, tag="gb")
            ube = state.tile([128, NB * E], BF16, tag="ube")
            seb = state.tile([E, E], BF16, tag="seb")
            xns = state.tile([E, E], F32, tag="xns")
            xb = state.tile([E, E], BF16, tag="xb")
            tb = state.tile([E, E], BF16, tag="tb")
            tf = state.tile([E, E], F32, tag="tf")
            zb = state.tile([128, TW], BF16, tag="zb")
            utb = state.tile([E, NB * 128], BF16, tag="utb")
            s_sb = state.tile([E, E], F32, tag="s_sb")
            dsb = state.tile([E, BS], F32, tag="dsb")
            wb = state.tile([E, BS], BF16, tag="wb")
            ysb = state.tile([128, NB * BS], F32, tag="ysb")

            Z = pz.tile([128, TW], F32, tag="Z")
            z3 = Z.rearrange("p (b w) -> p b w", w=W)
            zb3 = zb.rearrange("p (b w) -> p b w", w=W)

            with (
                tc.tile_pool(name="psolve", bufs=1, space="PSUM") as psolve,
                tc.tile_pool(name="pns", bufs=1, space="PSUM") as pns,
            ):
                # ---- input DMAs (HWDGE rings only; gpsimd boots ~6us late)
                # ---- + PE warm-up (HAM ramps with activity) ----
                nc.vector.memset(warm, 0.0)
                CMID = C_MT + NB * N // 2
                nc.sync.dma_start(b32, b32_d[:, :])
                nc.scalar.dma_start(b16[:, 0:C_MT], b16_d[:, 0:C_MT])
                nc.sync.dma_start(b16[:, C_MT:CMID], b16_d[:, C_MT:CMID])
                nc.scalar.dma_start(b16[:, CMID:C16], b16_d[:, CMID:C16])

                wps = psolve.tile([128, 128], F32, tag="wps")
                for _ in range(DUMMY_N):
                    nc.tensor.matmul(wps, warm, warm, start=True, stop=True)

                # ---- d_1 = 2*alpha * [x^T - p^T | A0^T] (bf16) ----
                nc.vector.tensor_sub(q, xt, pt)
                ga3 = ga.rearrange("p (b w) -> p b w", w=W)
                nc.scalar.activation(
                    ga3[:, :, 0:BS],
                    q.rearrange("p (b j) -> p b j", j=BS),
                    AF.Copy,
                    scale=S0,
                )
                nc.scalar.activation(
                    ga3[:, :, BS:W],
                    a0tb.rearrange("p (b e) -> p b e", e=E),
                    AF.Copy,
                    scale=S0,
                )

                # ---- solve: R += (-alpha Mt) @ d_k in PSUM (two half tiles);
                # ---- x = sum d_k in PSUM; d' = beta d + R on DVE ----
                Rq = [
                    psolve.tile([128, W], F32, tag=f"R{m}", name=f"Rq{m}")
                    for m in range(NB)
                ]

                def ns_emit(j):
                    # one pure-bf16 NS iteration: X <- 2X - X (S_e X)
                    t_ps = pns.tile([E, E], F32, tag="ns")
                    nc.tensor.matmul(t_ps, seb, xb, start=True, stop=True)
                    nc.scalar.activation(tb, t_ps, AF.Copy)
                    x2_ps = pns.tile([E, E], F32, tag="ns")
                    nc.tensor.matmul(x2_ps, xb, tb, start=True, stop=True)
                    nc.vector.scalar_tensor_tensor(
                        xb, xb, 2.0, x2_ps, op0=OP.mult, op1=OP.subtract
                    )

                for k in range(K - 1):
                    g_cur, g_nxt = (ga, gb) if k % 2 == 0 else (gb, ga)
                    for m in range(NB):
                        out = Rq[m][:, :]
                        for kb in range(NB):
                            if k == 0 and kb == m:
                                lhsT = mtg[:, m * 128 : (m + 1) * 128]
                            else:
                                lhsT = mtb[
                                    :, kb * N + m * 128 : kb * N + (m + 1) * 128
                                ]
                            # start marks the WHOLE psum bank pending-zero:
                            # exactly once per bank lifetime
                            nc.tensor.matmul(
                                out,
                                lhsT,
                                g_cur[:, kb * W : (kb + 1) * W],
                                start=(k == 0 and kb == 0),
                                stop=(k == K - 2 and kb == NB - 1),
                                skip_group_check=True,
                            )
                    # x accumulation (PE, one 320-row matmul vs identity)
                    nc.tensor.matmul(
                        Z, idb, g_cur, start=(k == 0), stop=False,
                        skip_group_check=True,
                    )
                    for h in range(NB):
                        sl = slice(h * W, (h + 1) * W)
                        eng = nc.vector
                        eng.scalar_tensor_tensor(
                            g_nxt[:, sl],
                            g_cur[:, sl],
                            BETA,
                            Rq[h][:, :],
                            op0=OP.mult,
                            op1=OP.add,
                        )
                    if k == S_EARLY:
                        # approximate S from the partial solution (U columns)
                        nc.scalar.activation(
                            ube.rearrange("p (b e) -> p b e", e=E),
                            z3[:, :, BS:W],
                            AF.Copy,
                        )
                        se_ps = pns.tile([E, E], F32, tag="ns")
                        for m in range(NB):
                            nc.tensor.matmul(
                                se_ps,
                                a0tb[:, m * E : (m + 1) * E],
                                ube[:, m * E : (m + 1) * E],
                                start=(m == 0),
                                stop=(m == NB - 1),
                            )
                        nc.vector.scalar_tensor_tensor(
                            xb, se_ps, B_NS, aeye, op0=OP.mult, op1=OP.add
                        )
                        nc.scalar.activation(seb, se_ps, AF.Copy)
                    if S_EARLY < k < S_EARLY + 1 + J_E:
                        ns_emit(k - S_EARLY - 1)

                # final increment accumulation
                g_fin = (ga, gb)[(K - 1) % 2]
                nc.tensor.matmul(
                    Z, idb, g_fin, start=False, stop=True, skip_group_check=True
                )
                for j in range(K - 1 - S_EARLY - 1, J_E):
                    ns_emit(j)

            # ---- tail ----
            # bf16 copy of the solution: U columns on ACT, batch cols on DVE
            nc.scalar.activation(zb3[:, :, BS:W], z3[:, :, BS:W], AF.Copy)
            nc.vector.tensor_copy(zb3[:, :, 0:BS], z3[:, :, 0:BS])
            with tc.tile_pool(name="ptail", bufs=1, space="PSUM") as ptail:
                sd_ps = ptail.tile([E, E + BS], F32, tag="sd")
                s_ps = sd_ps[:, 0:E]
                d_ps = sd_ps[:, E : E + BS]
                for m in range(NB):
                    nc.tensor.matmul(
                        s_ps,
                        a0tb[:, m * E : (m + 1) * E],
                        zb3[:, m, BS:W],
                        start=(m == 0),
                        stop=(m == NB - 1),
                    )
                for m in range(NB):
                    nc.tensor.matmul(
                        d_ps,
                        a0tb[:, m * E : (m + 1) * E],
                        zb3[:, m, 0:BS],
                        start=(m == 0),
                        stop=(m == NB - 1),
                    )
                nc.vector.tensor_copy(s_sb, s_ps)
                nc.vector.tensor_tensor(dsb, d_ps, cpos, op=OP.add)

                # U^T blocks for the final correction matmuls
                ut_ps = ptail.tile([E, NB * 128], BF16, tag="ut")
                for m in range(NB):
                    nc.tensor.transpose(
                        ut_ps[:, m * 128 : (m + 1) * 128], zb3[:, m, BS:W], idb
                    )
                nc.vector.tensor_copy(utb, ut_ps)

                # fp32 NS polish against the final S
                nc.vector.tensor_copy(xns, xb)
                for _ in range(J_F):
                    t_ps = ptail.tile([E, E], F32, tag="nsf")
                    nc.tensor.matmul(t_ps, s_sb, xns, start=True, stop=True)
                    nc.vector.tensor_copy(tf, t_ps)
                    x2_ps = ptail.tile([E, E], F32, tag="nsf")
                    nc.tensor.matmul(x2_ps, xns, tf, start=True, stop=True)
                    nc.vector.scalar_tensor_tensor(
                        xns, xns, 2.0, x2_ps, op0=OP.mult, op1=OP.subtract
                    )

                # W = Si D (negated); accumulate U W onto the x batch cols
                w_ps = ptail.tile([E, BS], F32, tag="nsf")
                nc.tensor.matmul(w_ps, xns, dsb, start=True, stop=True)
                nc.scalar.activation(wb, w_ps, AF.Copy, scale=-1.0)
                if dbg:
                    nc.sync.dma_start(s_dbg[:, :], s_sb)
                    nc.sync.dma_start(x_dbg[:, :], xns)
                    nc.gpsimd.dma_start(w_dbg[:, :], wb)

                for m in range(NB):
                    nc.tensor.matmul(
                        z3[:, m, 0:BS],
                        utb[:, m * 128 : (m + 1) * 128],
                        wb,
                        start=False,
                        stop=True,
                        skip_group_check=True,
                    )
                if bool(int(os.environ.get("KERNEL_PSUMDMA", "0"))):
                    ysb = z3[:, :, 0:BS]
                else:
                    nc.scalar.activation(
                        ysb.rearrange("p (b j) -> p b j", j=BS), z3[:, :, 0:BS],
                        AF.Copy,
                    )
                if dbg:
                    zc = state.tile([128, TW], F32, tag="zc")
                    nc.vector.tensor_copy(zc, Z)
                    nc.sync.dma_start(z_dbg[:, :], zc)
                nc.sync.dma_start(y_d[:, :], ysb)

    nc.compile()
    return nc


def _bf16(a):
    return np.asarray(a, dtype=np.float32).astype(BF)


def _prep_blobs(x, parms, M, A0, c):
    """Host-side layout/dtype marshalling (no input-data math)."""
    Mt = M + 2.0 * np.eye(N, dtype=np.float32)
    Mh = (-ALPHA * Mt).astype(np.float32)
    b16 = np.zeros((128, C16), dtype=BF)
    for m in range(NB):
        b16[:, C_A0 + m * E : C_A0 + (m + 1) * E] = _bf16(
            A0[:, m * 128 : (m + 1) * 128].T
        )
    b16[:, C_ID : C_ID + 128] = _bf16(np.eye(128, dtype=np.float32))
    for m in range(NB):
        blk = Mh[m * 128 : (m + 1) * 128, m * 128 : (m + 1) * 128] + np.eye(
            128, dtype=np.float32
        )
        b16[:, C_MG + m * 128 : C_MG + (m + 1) * 128] = _bf16(blk)
    for kb in range(NB):
        b16[:, C_MT + kb * N : C_MT + (kb + 1) * N] = _bf16(
            Mh[kb * 128 : (kb + 1) * 128, :]
        )

    base = np.zeros((128, C32), dtype=np.float32)
    base[0:E, F_AE : F_AE + E] = A_NS * np.eye(E, dtype=np.float32)
    base[0:E, F_CP : F_CP + BS] = np.repeat(c.reshape(E, 1), BS, axis=1)
    b32s = []
    for i in range(NCORES):
        b32 = base.copy()
        xs = x[i * BS : (i + 1) * BS]
        ps = parms[i * BS : (i + 1) * BS]
        for m in range(NB):
            b32[:, F_XT + m * BS : F_XT + (m + 1) * BS] = xs[
                :, m * 128 : (m + 1) * 128
            ].T
            b32[:, F_PT + m * BS : F_PT + (m + 1) * BS] = ps[
                :, m * 128 : (m + 1) * 128
            ].T
        b32s.append(np.ascontiguousarray(b32))
    return np.ascontiguousarray(b16), b32s


def _ensure_axon_ntff_hook():
    """Provide antenv.axon_hooks if the image lacks it (profiling only)."""
    try:
        import antenv.axon_hooks  # noqa: F401

        return
    except ImportError:
        pass
    import contextlib
    import ctypes
    import types

    hook = None
    so_path = "/opt/axon/libaxon_pjrt.so"
    if os.path.exists(so_path):
        lib = ctypes.CDLL(so_path)
        if hasattr(lib, "axon_start_nrt_profile"):
            lib.axon_start_nrt_profile.argtypes = [
                ctypes.POINTER(ctypes.c_int64),
                ctypes.c_size_t,
            ]
            lib.axon_start_nrt_profile.restype = ctypes.c_int64
            lib.axon_stop_nrt_profile.argtypes = [ctypes.c_char_p]
            lib.axon_stop_nrt_profile.restype = ctypes.c_int64

            @contextlib.contextmanager
            def _hook(output_dir, device_ids):
                import jax

                jax.devices()
                if device_ids:
                    ids = (ctypes.c_int64 * len(device_ids))(*device_ids)
                    rc = lib.axon_start_nrt_profile(ids, len(device_ids))
                else:
                    rc = lib.axon_start_nrt_profile(None, 0)
                if rc != 0:
                    raise RuntimeError(f"axon_start_nrt_profile rc={rc}")
                try:
                    yield
                finally:
                    n = lib.axon_stop_nrt_profile(str(output_dir).encode())
                    print(f"ntff profile: {n} file(s) -> {output_dir}")

            hook = _hook

    mod = types.ModuleType("antenv.axon_hooks")
    mod.get_axon_ntff_profile_hook = lambda: hook
    mod.set_axon_ntff_profile_hook = lambda h: None
    sys.modules["antenv.axon_hooks"] = mod


_NC_CACHE = {}


def kernel(x, parms, M, A0, B0=None, c=None, **_unused):
    x = np.ascontiguousarray(x, dtype=np.float32)
    parms = np.ascontiguousarray(parms, dtype=np.float32)
    M = np.ascontiguousarray(M, dtype=np.float32)
    A0 = np.ascontiguousarray(A0, dtype=np.float32)
    c = np.ascontiguousarray(c, dtype=np.float32).reshape(E)

    dbg = bool(int(os.environ.get("KERNEL_DEBUG", "0")))
    if dbg not in _NC_CACHE:
        _NC_CACHE[dbg] = build_bass(dbg)
    nc = _NC_CACHE[dbg]

    b16, b32s = _prep_blobs(x, parms, M, A0, c)
    in_maps = [{"b16": b16, "b32": b32s[i]} for i in range(NCORES)]

    trace = bool(int(os.environ.get("KERNEL_TRACE", "0")))
    if trace:
        _ensure_axon_ntff_hook()
    res = run_bass_kernel_spmd(
        nc, in_maps, core_ids=list(range(NCORES)), trace=trace
    )
    LAST_RUN["exec_time_ns"] = res.exec_time_ns
    LAST_RUN["mean_exec_time_ns"] = res.mean_exec_time_ns
    LAST_RUN["trace"] = res.instructions_and_trace
    LAST_RUN["profile_json"] = res.profile_json
    LAST_RUN["debug"] = {
        k: v for k, v in res.results[0].items() if k.startswith("dbg_")
    }
    out = np.empty((B, N), dtype=np.float32)
    for i, r in enumerate(res.results):
        yc = np.asarray(r["y"], dtype=np.float32)  # [128, m*BS+j]
        for m in range(NB):
            out[i * BS : (i + 1) * BS, m * 128 : (m + 1) * 128] = yc[
                :, m * BS : (m + 1) * BS
            ].T
    return out


# revision 24
# speedup vs baseline: 1.0235x; 1.0235x over previous
"""Trainium2 Bass kernel for nn_EqualityConstrainedQuadratic.

Mathematical structure (verified against the reference):
  - The per-sample KKT matrices are identical across the batch (Hessian of
    f is M, jacrev(F) wrt x is A0, b = -F(0,0) = -c), so the batch shares one
    576x576 saddle solve with per-sample right-hand sides; B0 never matters.
  - With H = M/2 + I and Mt = M + 2I (spectrum ~[2, 7]):
        y = Y1 - U Si (A0 Y1 + c),  [Y1|U] = Z,  Mt Z = 2 [x-parms | A0^T],
        S = A0 U, Si = S^-1.
  - Single bf16 heavy-ball solve (K=7 iterations, constant coefficients
    alpha/beta), no iterative refinement: the bf16(M) quantization floor is
    ~3-4e-3 relative, well under the 2e-2 gate.

Execution design (per core; data parallel over batch, 16 samples/core):
  - Heavy-ball in "increment" form: d_{k+1} = beta d_k + alpha r_k,
    x_{k+1} = x_k + d_{k+1}, r_{k+1} = r_k - Mt d_{k+1}.  The scaled
    residual R = alpha*r lives IN PSUM as a running matmul accumulation
    R += (-alpha Mt) @ d_k with the matrix scale folded on the host, so the
    whole per-sweep update is ONE single-scalar DVE STT per half:
    d' = beta*d + R (in1 read straight from PSUM).  No ACT in the loop.
  - The solution x = sum_k d_k accumulates in a second PSUM bank via an
    identity matmul.  The first sweep uses modified diagonal blocks
    bf16(I - alpha Mt), which folds the R init in (alpha r_0 = d_1).
  - R is split into two PSUM tiles (halves) so the DVE updates don't wait
    on the full 16-matmul sweep (Tile tracks deps per tile).
  - PSUM accumulation gotcha: start_tensor_calc marks the WHOLE 2KB bank
    pending-zero, so each bank gets start=True exactly once.
  - The Schur inverse runs via Newton-Schulz on an EARLY approximate S
    (from the partially converged U) with a quadratic polynomial init,
    overlapped with the remaining sweeps; one fp32 NS polish against the
    final S in the tail.
  - Output is written in column layout and transposed on the host;
    host pre-marshals layouts/dtypes only (transposed tiles, bf16 casts of
    scaled Mt, identity); all input-data math runs on device.
  - PE is kept warm through the input DMA with dummy matmuls (HAM ramps to
    2.4 GHz after ~3.4us of activity; all DMA on the two HWDGE rings since
    gpsimd/SWDGE boots ~6us late).
"""

import os
import sys

import numpy as np

for _p in ("/root/.axon_site", "/root/.axon_site/_ro/trn_rl_repo"):
    if os.path.isdir(_p) and _p not in sys.path:
        sys.path.append(_p)

import ml_dtypes

import concourse.mybir as mybir
from concourse import bacc
from concourse.bass_utils import run_bass_kernel_spmd
from concourse.tile import TileContext

F32 = mybir.dt.float32
BF16 = mybir.dt.bfloat16
OP = mybir.AluOpType
AF = mybir.ActivationFunctionType
BF = ml_dtypes.bfloat16

# problem shape (hardcoded per contract)
B, N, E = 128, 512, 64
NCORES = 8
BS = B // NCORES  # 16 samples per core
NB = N // 128  # 4 row blocks
W = BS + E  # 80 state columns per core
TW = NB * W  # 320

# spectral bounds of Mt = M + 2I -> heavy-ball coefficients
A_LO, A_HI = 2.0, 7.5
ALPHA = (2.0 / (A_HI**0.5 + A_LO**0.5)) ** 2
BETA = ((A_HI / A_LO) ** 0.5 - 1.0) ** 2 / ((A_HI / A_LO) ** 0.5 + 1.0) ** 2

K = int(os.environ.get("KERNEL_K", "6"))  # heavy-ball iterations (K-1 sweeps)
S_EARLY = max(1, K - 4)  # build approximate S from the partial solution
J_E = 2  # NS bf16 iterations on early S
J_F = 1  # NS fp32 polish iterations on final S
A_NS, B_NS = 2.5, -1.023  # NS quadratic init X0 = a I + b S
DUMMY_N = int(os.environ.get("KERNEL_DUMMY", "42"))  # PE warm-up matmuls

S0 = 2.0 * ALPHA  # d_1 = alpha * 2 * [q | A0^T]

# bf16 blob column layout: [a0tb | idb | mtg | mtb]
C_A0 = 0
C_ID = C_A0 + NB * E  # 256
C_MG = C_ID + 128  # 384
C_MT = C_MG + NB * 128  # 896
C16 = C_MT + NB * N  # 2944
# f32 blob column layout: [xt | pt | aeye | cpos]
F_XT = 0
F_PT = F_XT + NB * BS  # 64
F_AE = F_PT + NB * BS  # 128
F_CP = F_AE + E  # 192
C32 = F_CP + BS  # 208

LAST_RUN = {}


def build_bass(dbg=False):
    nc = bacc.Bacc("TRN2", target_bir_lowering=False)

    b16_d = nc.dram_tensor("b16", [128, C16], BF16, kind="ExternalInput")
    b32_d = nc.dram_tensor("b32", [128, C32], F32, kind="ExternalInput")
    # y in column layout [128, m*BS+j]; the host transposes back
    y_d = nc.dram_tensor("y", [128, NB * BS], F32, kind="ExternalOutput")
    if dbg:
        z_dbg = nc.dram_tensor("dbg_z", [128, TW], F32, kind="ExternalOutput")
        s_dbg = nc.dram_tensor("dbg_s", [E, E], F32, kind="ExternalOutput")
        x_dbg = nc.dram_tensor("dbg_x", [E, E], F32, kind="ExternalOutput")
        w_dbg = nc.dram_tensor("dbg_w", [E, BS], F32, kind="ExternalOutput")

    with TileContext(nc) as tc:
        with (
            tc.tile_pool(name="consts", bufs=1) as consts,
            tc.tile_pool(name="state", bufs=1) as state,
            tc.tile_pool(name="pz", bufs=1, space="PSUM") as pz,
        ):
            b16 = consts.tile([128, C16], BF16, tag="b16")
            b32 = consts.tile([128, C32], F32, tag="b32")
            a0tb = b16[:, C_A0:C_ID]
            idb = b16[:, C_ID:C_MG]
            mtg = b16[:, C_MG:C_MT]
            mtb = b16[:, C_MT:C16]
            xt = b32[:, F_XT:F_PT]
            pt = b32[:, F_PT:F_AE]
            aeye = b32[0:E, F_AE:F_CP]
            cpos = b32[0:E, F_CP:C32]

            warm = consts.tile([128, 128], BF16, tag="warm")
            q = state.tile([128, NB * BS], F32, tag="q")
            ga = state.tile([128, TW], BF16, tag="ga")
            gb = state.tile([128, TW], BF16, tag="gb")
            ube = state.tile([128, NB * E], BF16, tag="ube")
            seb = state.tile([E, E], BF16, tag="seb")
            xns = state.tile([E, E], F32, tag="xns")
            xb = state.tile([E, E], BF16, tag="xb")
            tb = state.tile([E, E], BF16, tag="tb")
            tf = state.tile([E, E], F32, tag="tf")
            zb = state.tile([128, TW], BF16, tag="zb")
            utb = state.tile([E, NB * 128], BF16, tag="utb")
            s_sb = state.tile([E, E], F32, tag="s_sb")
            dsb = state.tile([E, BS], F32, tag="dsb")
            wb = state.tile([E, BS], BF16, tag="wb")
            ysb = state.tile([128, NB * BS], F32, tag="ysb")

            Z = pz.tile([128, TW], F32, tag="Z")
            z3 = Z.rearrange("p (b w) -> p b w", w=W)
            zb3 = zb.rearrange("p (b w) -> p b w", w=W)

            with (
                tc.tile_pool(name="psolve", bufs=1, space="PSUM") as psolve,
                tc.tile_pool(name="pns", bufs=1, space="PSUM") as pns,
            ):
                # ---- input DMAs (HWDGE rings only; gpsimd boots ~6us late)
                # ---- + PE warm-up (HAM ramps with activity) ----
                nc.vector.memset(warm, 0.0)
                CMID = C_MT + NB * N // 2
                nc.sync.dma_start(b32, b32_d[:, :])
                nc.scalar.dma_start(b16[:, 0:C_MT], b16_d[:, 0:C_MT])
                nc.sync.dma_start(b16[:, C_MT:CMID], b16_d[:, C_MT:CMID])
                nc.scalar.dma_start(b16[:, CMID:C16], b16_d[:, CMID:C16])

                wps = psolve.tile([128, 128], F32, tag="wps")
                for _ in range(DUMMY_N):
                    nc.tensor.matmul(wps, warm, warm, start=True, stop=True)

                # ---- d_1 = 2*alpha * [x^T - p^T | A0^T] (bf16) ----
                nc.vector.tensor_sub(q, xt, pt)
                ga3 = ga.rearrange("p (b w) -> p b w", w=W)
                nc.scalar.activation(
                    ga3[:, :, 0:BS],
                    q.rearrange("p (b j) -> p b j", j=BS),
                    AF.Copy,
                    scale=S0,
                )
                nc.scalar.activation(
                    ga3[:, :, BS:W],
                    a0tb.rearrange("p (b e) -> p b e", e=E),
                    AF.Copy,
                    scale=S0,
                )

                # ---- solve: R += (-alpha Mt) @ d_k in PSUM (two half tiles);
                # ---- x = sum d_k in PSUM; d' = beta d + R on DVE ----
                Rq = [
                    psolve.tile([128, W], F32, tag=f"R{m}", name=f"Rq{m}")
                    for m in range(NB)
                ]

                def ns_emit(j):
                    # one pure-bf16 NS iteration: X <- 2X - X (S_e X)
                    t_ps = pns.tile([E, E], F32, tag="ns")
                    nc.tensor.matmul(t_ps, seb, xb, start=True, stop=True)
                    nc.scalar.activation(tb, t_ps, AF.Copy)
                    x2_ps = pns.tile([E, E], F32, tag="ns")
                    nc.tensor.matmul(x2_ps, xb, tb, start=True, stop=True)
                    nc.vector.scalar_tensor_tensor(
                        xb, xb, 2.0, x2_ps, op0=OP.mult, op1=OP.subtract
                    )

                for k in range(K - 1):
                    g_cur, g_nxt = (ga, gb) if k % 2 == 0 else (gb, ga)
                    for m in range(NB):
                        out = Rq[m][:, :]
                        for kb in range(NB):
                            if k == 0 and kb == m:
                                lhsT = mtg[:, m * 128 : (m + 1) * 128]
                            else:
                                lhsT = mtb[
                                    :, kb * N + m * 128 : kb * N + (m + 1) * 128
                                ]
                            # start marks the WHOLE psum bank pending-zero:
                            # exactly once per bank lifetime
                            nc.tensor.matmul(
                                out,
                                lhsT,
                                g_cur[:, kb * W : (kb + 1) * W],
                                start=(k == 0 and kb == 0),
                                stop=(k == K - 2 and kb == NB - 1),
                                skip_group_check=True,
                            )
                    # x accumulation (PE, one 320-row matmul vs identity)
                    nc.tensor.matmul(
                        Z, idb, g_cur, start=(k == 0), stop=False,
                        skip_group_check=True,
                    )
                    for h in range(NB):
                        sl = slice(h * W, (h + 1) * W)
                        eng = nc.vector
                        eng.scalar_tensor_tensor(
                            g_nxt[:, sl],
                            g_cur[:, sl],
                            BETA,
                            Rq[h][:, :],
                            op0=OP.mult,
                            op1=OP.add,
                        )
                    if k == S_EARLY:
                        # approximate S from the partial solution (U columns)
                        nc.scalar.activation(
                            ube.rearrange("p (b e) -> p b e", e=E),
                            z3[:, :, BS:W],
                            AF.Copy,
                        )
                        se_ps = pns.tile([E, E], F32, tag="ns")
                        for m in range(NB):
                            nc.tensor.matmul(
                                se_ps,
                                a0tb[:, m * E : (m + 1) * E],
                                ube[:, m * E : (m + 1) * E],
                                start=(m == 0),
                                stop=(m == NB - 1),
                            )
                        nc.vector.scalar_tensor_tensor(
                            xb, se_ps, B_NS, aeye, op0=OP.mult, op1=OP.add
                        )
                        nc.scalar.activation(seb, se_ps, AF.Copy)
                    if k == S_EARLY + 1:
                        ns_emit(0)

                # final increment accumulation
                g_fin = (ga, gb)[(K - 1) % 2]
                nc.tensor.matmul(
                    Z, idb, g_fin, start=False, stop=True, skip_group_check=True
                )
                # zb copies BEFORE the remaining NS emission so the ACT queue
                # is not head-blocked by the NS chain
                nc.scalar.activation(zb3[:, :, BS:W], z3[:, :, BS:W], AF.Copy)
                nc.vector.tensor_copy(zb3[:, :, 0:BS], z3[:, :, 0:BS])
                for j in range(1, J_E):
                    ns_emit(j)

            # ---- tail ----
            with tc.tile_pool(name="ptail", bufs=1, space="PSUM") as ptail:
                sd_ps = ptail.tile([E, E + BS], F32, tag="sd")
                s_ps = sd_ps[:, 0:E]
                d_ps = sd_ps[:, E : E + BS]
                for m in range(NB):
                    nc.tensor.matmul(
                        s_ps,
                        a0tb[:, m * E : (m + 1) * E],
                        zb3[:, m, BS:W],
                        start=(m == 0),
                        stop=(m == NB - 1),
                    )
                nc.vector.tensor_copy(s_sb, s_ps)
                for m in range(NB):
                    nc.tensor.matmul(
                        d_ps,
                        a0tb[:, m * E : (m + 1) * E],
                        zb3[:, m, 0:BS],
                        start=(m == 0),
                        stop=(m == NB - 1),
                    )
                nc.vector.tensor_tensor(dsb, d_ps, cpos, op=OP.add)

                # U^T blocks for the final correction matmuls
                ut_ps = ptail.tile([E, NB * 128], BF16, tag="ut")
                for m in range(NB):
                    nc.tensor.transpose(
                        ut_ps[:, m * 128 : (m + 1) * 128], zb3[:, m, BS:W], idb
                    )
                nc.vector.tensor_copy(utb, ut_ps)

                # fp32 NS polish against the final S
                nc.vector.tensor_copy(xns, xb)
                for _ in range(J_F):
                    t_ps = ptail.tile([E, E], F32, tag="nsf")
                    nc.tensor.matmul(t_ps, s_sb, xns, start=True, stop=True)
                    nc.vector.tensor_copy(tf, t_ps)
                    x2_ps = ptail.tile([E, E], F32, tag="nsf")
                    nc.tensor.matmul(x2_ps, xns, tf, start=True, stop=True)
                    nc.vector.scalar_tensor_tensor(
                        xns, xns, 2.0, x2_ps, op0=OP.mult, op1=OP.subtract
                    )

                # W = Si D (negated); accumulate U W onto the x batch cols
                w_ps = ptail.tile([E, BS], F32, tag="nsf")
                nc.tensor.matmul(w_ps, xns, dsb, start=True, stop=True)
                nc.scalar.activation(wb, w_ps, AF.Copy, scale=-1.0)
                if dbg:
                    nc.sync.dma_start(s_dbg[:, :], s_sb)
                    nc.sync.dma_start(x_dbg[:, :], xns)
                    nc.gpsimd.dma_start(w_dbg[:, :], wb)

                for m in range(NB):
                    nc.tensor.matmul(
                        z3[:, m, 0:BS],
                        utb[:, m * 128 : (m + 1) * 128],
                        wb,
                        start=False,
                        stop=True,
                        skip_group_check=True,
                    )
                nc.vector.tensor_copy(
                    ysb.rearrange("p (b j) -> p b j", j=BS), z3[:, :, 0:BS]
                )
                if dbg:
                    zc = state.tile([128, TW], F32, tag="zc")
                    nc.vector.tensor_copy(zc, Z)
                    nc.sync.dma_start(z_dbg[:, :], zc)
                nc.sync.dma_start(y_d[:, :], ysb)

    nc.compile()
    return nc


def _bf16(a):
    return np.asarray(a, dtype=np.float32).astype(BF)


def _prep_blobs(x, parms, M, A0, c):
    """Host-side layout/dtype marshalling (no input-data math)."""
    Mt = M + 2.0 * np.eye(N, dtype=np.float32)
    Mh = (-ALPHA * Mt).astype(np.float32)
    b16 = np.zeros((128, C16), dtype=BF)
    for m in range(NB):
        b16[:, C_A0 + m * E : C_A0 + (m + 1) * E] = _bf16(
            A0[:, m * 128 : (m + 1) * 128].T
        )
    b16[:, C_ID : C_ID + 128] = _bf16(np.eye(128, dtype=np.float32))
    for m in range(NB):
        blk = Mh[m * 128 : (m + 1) * 128, m * 128 : (m + 1) * 128] + np.eye(
            128, dtype=np.float32
        )
        b16[:, C_MG + m * 128 : C_MG + (m + 1) * 128] = _bf16(blk)
    for kb in range(NB):
        b16[:, C_MT + kb * N : C_MT + (kb + 1) * N] = _bf16(
            Mh[kb * 128 : (kb + 1) * 128, :]
        )

    base = np.zeros((128, C32), dtype=np.float32)
    base[0:E, F_AE : F_AE + E] = A_NS * np.eye(E, dtype=np.float32)
    base[0:E, F_CP : F_CP + BS] = np.repeat(c.reshape(E, 1), BS, axis=1)
    b32s = []
    for i in range(NCORES):
        b32 = base.copy()
        xs = x[i * BS : (i + 1) * BS]
        ps = parms[i * BS : (i + 1) * BS]
        for m in range(NB):
            b32[:, F_XT + m * BS : F_XT + (m + 1) * BS] = xs[
                :, m * 128 : (m + 1) * 128
            ].T
            b32[:, F_PT + m * BS : F_PT + (m + 1) * BS] = ps[
                :, m * 128 : (m + 1) * 128
            ].T
        b32s.append(np.ascontiguousarray(b32))
    return np.ascontiguousarray(b16), b32s


def _ensure_axon_ntff_hook():
    """Provide antenv.axon_hooks if the image lacks it (profiling only)."""
    try:
        import antenv.axon_hooks  # noqa: F401

        return
    except ImportError:
        pass
    import contextlib
    import ctypes
    import types

    hook = None
    so_path = "/opt/axon/libaxon_pjrt.so"
    if os.path.exists(so_path):
        lib = ctypes.CDLL(so_path)
        if hasattr(lib, "axon_start_nrt_profile"):
            lib.axon_start_nrt_profile.argtypes = [
                ctypes.POINTER(ctypes.c_int64),
                ctypes.c_size_t,
            ]
            lib.axon_start_nrt_profile.restype = ctypes.c_int64
            lib.axon_stop_nrt_profile.argtypes = [ctypes.c_char_p]
            lib.axon_stop_nrt_profile.restype = ctypes.c_int64

            @contextlib.contextmanager
            def _hook(output_dir, device_ids):
                import jax

                jax.devices()
                if device_ids:
                    ids = (ctypes.c_int64 * len(device_ids))(*device_ids)
                    rc = lib.axon_start_nrt_profile(ids, len(device_ids))
                else:
                    rc = lib.axon_start_nrt_profile(None, 0)
                if rc != 0:
                    raise RuntimeError(f"axon_start_nrt_profile rc={rc}")
                try:
                    yield
                finally:
                    n = lib.axon_stop_nrt_profile(str(output_dir).encode())
                    print(f"ntff profile: {n} file(s) -> {output_dir}")

            hook = _hook

    mod = types.ModuleType("antenv.axon_hooks")
    mod.get_axon_ntff_profile_hook = lambda: hook
    mod.set_axon_ntff_profile_hook = lambda h: None
    sys.modules["antenv.axon_hooks"] = mod


_NC_CACHE = {}


def kernel(x, parms, M, A0, B0=None, c=None, **_unused):
    x = np.ascontiguousarray(x, dtype=np.float32)
    parms = np.ascontiguousarray(parms, dtype=np.float32)
    M = np.ascontiguousarray(M, dtype=np.float32)
    A0 = np.ascontiguousarray(A0, dtype=np.float32)
    c = np.ascontiguousarray(c, dtype=np.float32).reshape(E)

    dbg = bool(int(os.environ.get("KERNEL_DEBUG", "0")))
    if dbg not in _NC_CACHE:
        _NC_CACHE[dbg] = build_bass(dbg)
    nc = _NC_CACHE[dbg]

    b16, b32s = _prep_blobs(x, parms, M, A0, c)
    in_maps = [{"b16": b16, "b32": b32s[i]} for i in range(NCORES)]

    trace = bool(int(os.environ.get("KERNEL_TRACE", "0")))
    if trace:
        _ensure_axon_ntff_hook()
    res = run_bass_kernel_spmd(
        nc, in_maps, core_ids=list(range(NCORES)), trace=trace
    )
    LAST_RUN["exec_time_ns"] = res.exec_time_ns
    LAST_RUN["mean_exec_time_ns"] = res.mean_exec_time_ns
    LAST_RUN["trace"] = res.instructions_and_trace
    LAST_RUN["profile_json"] = res.profile_json
    LAST_RUN["debug"] = {
        k: v for k, v in res.results[0].items() if k.startswith("dbg_")
    }
    out = np.empty((B, N), dtype=np.float32)
    for i, r in enumerate(res.results):
        yc = np.asarray(r["y"], dtype=np.float32)  # [128, m*BS+j]
        for m in range(NB):
            out[i * BS : (i + 1) * BS, m * 128 : (m + 1) * 128] = yc[
                :, m * BS : (m + 1) * BS
            ].T
    return out


# revision 25
# speedup vs baseline: 1.0275x; 1.0040x over previous
"""Trainium2 Bass kernel for nn_EqualityConstrainedQuadratic.

Mathematical structure (verified against the reference):
  - The per-sample KKT matrices are identical across the batch (Hessian of
    f is M, jacrev(F) wrt x is A0, b = -F(0,0) = -c), so the batch shares one
    576x576 saddle solve with per-sample right-hand sides; B0 never matters.
  - With H = M/2 + I and Mt = M + 2I (spectrum ~[2, 7]):
        y = Y1 - U Si (A0 Y1 + c),  [Y1|U] = Z,  Mt Z = 2 [x-parms | A0^T],
        S = A0 U, Si = S^-1.
  - Single bf16 heavy-ball solve (K=7 iterations, constant coefficients
    alpha/beta), no iterative refinement: the bf16(M) quantization floor is
    ~3-4e-3 relative, well under the 2e-2 gate.

Execution design (per core; data parallel over batch, 16 samples/core):
  - Heavy-ball in "increment" form: d_{k+1} = beta d_k + alpha r_k,
    x_{k+1} = x_k + d_{k+1}, r_{k+1} = r_k - Mt d_{k+1}.  The scaled
    residual R = alpha*r lives IN PSUM as a running matmul accumulation
    R += (-alpha Mt) @ d_k with the matrix scale folded on the host, so the
    whole per-sweep update is ONE single-scalar DVE STT per half:
    d' = beta*d + R (in1 read straight from PSUM).  No ACT in the loop.
  - The solution x = sum_k d_k accumulates in a second PSUM bank via an
    identity matmul.  The first sweep uses modified diagonal blocks
    bf16(I - alpha Mt), which folds the R init in (alpha r_0 = d_1).
  - R is split into two PSUM tiles (halves) so the DVE updates don't wait
    on the full 16-matmul sweep (Tile tracks deps per tile).
  - PSUM accumulation gotcha: start_tensor_calc marks the WHOLE 2KB bank
    pending-zero, so each bank gets start=True exactly once.
  - The Schur inverse runs via Newton-Schulz on an EARLY approximate S
    (from the partially converged U) with a quadratic polynomial init,
    overlapped with the remaining sweeps; one fp32 NS polish against the
    final S in the tail.
  - Output is written in column layout and transposed on the host;
    host pre-marshals layouts/dtypes only (transposed tiles, bf16 casts of
    scaled Mt, identity); all input-data math runs on device.
  - PE is kept warm through the input DMA with dummy matmuls (HAM ramps to
    2.4 GHz after ~3.4us of activity; all DMA on the two HWDGE rings since
    gpsimd/SWDGE boots ~6us late).
"""

import os
import sys

import numpy as np

for _p in ("/root/.axon_site", "/root/.axon_site/_ro/trn_rl_repo"):
    if os.path.isdir(_p) and _p not in sys.path:
        sys.path.append(_p)

import ml_dtypes

import concourse.mybir as mybir
from concourse import bacc
from concourse.bass_utils import run_bass_kernel_spmd
from concourse.tile import TileContext

F32 = mybir.dt.float32
BF16 = mybir.dt.bfloat16
OP = mybir.AluOpType
AF = mybir.ActivationFunctionType
BF = ml_dtypes.bfloat16

# problem shape (hardcoded per contract)
B, N, E = 128, 512, 64
NCORES = 8
BS = B // NCORES  # 16 samples per core
NB = N // 128  # 4 row blocks
W = BS + E  # 80 state columns per core
TW = NB * W  # 320

# spectral bounds of Mt = M + 2I -> heavy-ball coefficients
A_LO, A_HI = 2.0, 7.5
ALPHA = (2.0 / (A_HI**0.5 + A_LO**0.5)) ** 2
BETA = ((A_HI / A_LO) ** 0.5 - 1.0) ** 2 / ((A_HI / A_LO) ** 0.5 + 1.0) ** 2

K = int(os.environ.get("KERNEL_K", "6"))  # heavy-ball iterations (K-1 sweeps)
S_EARLY = max(1, K - 4)  # build approximate S from the partial solution
J_E = 2  # NS bf16 iterations on early S
J_F = 1  # NS fp32 polish iterations on final S
A_NS, B_NS = 2.5, -1.023  # NS quadratic init X0 = a I + b S
DUMMY_N = int(os.environ.get("KERNEL_DUMMY", "42"))  # PE warm-up matmuls

S0 = 2.0 * ALPHA  # d_1 = alpha * 2 * [q | A0^T]

# bf16 blob column layout: [a0tb | idb | mtg | mtb]
C_A0 = 0
C_ID = C_A0 + NB * E  # 256
C_MG = C_ID + 128  # 384
C_MT = C_MG + NB * 128  # 896
C16 = C_MT + NB * N  # 2944
# f32 blob column layout: [xt | pt | aeye | cpos]
F_XT = 0
F_PT = F_XT + NB * BS  # 64
F_AE = F_PT + NB * BS  # 128
F_CP = F_AE + E  # 192
C32 = F_CP + BS  # 208

LAST_RUN = {}


def build_bass(dbg=False):
    nc = bacc.Bacc("TRN2", target_bir_lowering=False)

    b16_d = nc.dram_tensor("b16", [128, C16], BF16, kind="ExternalInput")
    b32_d = nc.dram_tensor("b32", [128, C32], F32, kind="ExternalInput")
    # y in column layout [128, m*BS+j]; the host transposes back
    y_d = nc.dram_tensor("y", [128, NB * BS], F32, kind="ExternalOutput")
    if dbg:
        z_dbg = nc.dram_tensor("dbg_z", [128, TW], F32, kind="ExternalOutput")
        s_dbg = nc.dram_tensor("dbg_s", [E, E], F32, kind="ExternalOutput")
        x_dbg = nc.dram_tensor("dbg_x", [E, E], F32, kind="ExternalOutput")
        w_dbg = nc.dram_tensor("dbg_w", [E, BS], F32, kind="ExternalOutput")

    with TileContext(nc) as tc:
        with (
            tc.tile_pool(name="consts", bufs=1) as consts,
            tc.tile_pool(name="state", bufs=1) as state,
            tc.tile_pool(name="pz", bufs=1, space="PSUM") as pz,
        ):
            b16 = consts.tile([128, C16], BF16, tag="b16")
            b32 = consts.tile([128, C32], F32, tag="b32")
            a0tb = b16[:, C_A0:C_ID]
            idb = b16[:, C_ID:C_MG]
            mtg = b16[:, C_MG:C_MT]
            mtb = b16[:, C_MT:C16]
            xt = b32[:, F_XT:F_PT]
            pt = b32[:, F_PT:F_AE]
            aeye = b32[0:E, F_AE:F_CP]
            cpos = b32[0:E, F_CP:C32]

            warm = consts.tile([128, 128], BF16, tag="warm")
            q = state.tile([128, NB * BS], F32, tag="q")
            ga = state.tile([128, TW], BF16, tag="ga")
            gb = state.tile([128, TW], BF16, tag="gb")
            ube = state.tile([128, NB * E], BF16, tag="ube")
            seb = state.tile([E, E], BF16, tag="seb")
            xns = state.tile([E, E], F32, tag="xns")
            xb = state.tile([E, E], BF16, tag="xb")
            tb = state.tile([E, E], BF16, tag="tb")
            tf = state.tile([E, E], F32, tag="tf")
            zb = state.tile([128, TW], BF16, tag="zb")
            utb = state.tile([E, NB * 128], BF16, tag="utb")
            s_sb = state.tile([E, E], F32, tag="s_sb")
            dsb = state.tile([E, BS], F32, tag="dsb")
            wb = state.tile([E, BS], BF16, tag="wb")
            ysb = state.tile([128, NB * BS], F32, tag="ysb")

            Z = pz.tile([128, TW], F32, tag="Z")
            z3 = Z.rearrange("p (b w) -> p b w", w=W)
            zb3 = zb.rearrange("p (b w) -> p b w", w=W)

            with (
                tc.tile_pool(name="psolve", bufs=1, space="PSUM") as psolve,
                tc.tile_pool(name="pns", bufs=1, space="PSUM") as pns,
            ):
                # ---- input DMAs (HWDGE rings only; gpsimd boots ~6us late)
                # ---- + PE warm-up (HAM ramps with activity) ----
                nc.vector.memset(warm, 0.0)
                CMID = C_MT + NB * N // 2
                nc.sync.dma_start(b32, b32_d[:, :])
                nc.scalar.dma_start(b16[:, 0:C_MT], b16_d[:, 0:C_MT])
                nc.sync.dma_start(b16[:, C_MT:CMID], b16_d[:, C_MT:CMID])
                nc.scalar.dma_start(b16[:, CMID:C16], b16_d[:, CMID:C16])

                wps = psolve.tile([128, 128], F32, tag="wps")
                for _ in range(DUMMY_N):
                    nc.tensor.matmul(wps, warm, warm, start=True, stop=True)

                # ---- d_1 = 2*alpha * [x^T - p^T | A0^T] (bf16) ----
                nc.vector.tensor_sub(q, xt, pt)
                ga3 = ga.rearrange("p (b w) -> p b w", w=W)
                nc.scalar.activation(
                    ga3[:, :, 0:BS],
                    q.rearrange("p (b j) -> p b j", j=BS),
                    AF.Copy,
                    scale=S0,
                )
                nc.scalar.activation(
                    ga3[:, :, BS:W],
                    a0tb.rearrange("p (b e) -> p b e", e=E),
                    AF.Copy,
                    scale=S0,
                )

                # ---- solve: R += (-alpha Mt) @ d_k in PSUM (two half tiles);
                # ---- x = sum d_k in PSUM; d' = beta d + R on DVE ----
                Rq = [
                    psolve.tile([128, W], F32, tag=f"R{m}", name=f"Rq{m}")
                    for m in range(NB)
                ]

                def ns_emit(j):
                    # one pure-bf16 NS iteration: X <- 2X - X (S_e X)
                    t_ps = pns.tile([E, E], F32, tag="ns")
                    nc.tensor.matmul(t_ps, seb, xb, start=True, stop=True)
                    nc.scalar.activation(tb, t_ps, AF.Copy)
                    x2_ps = pns.tile([E, E], F32, tag="ns")
                    nc.tensor.matmul(x2_ps, xb, tb, start=True, stop=True)
                    nc.vector.scalar_tensor_tensor(
                        xb, xb, 2.0, x2_ps, op0=OP.mult, op1=OP.subtract
                    )

                for k in range(K - 1):
                    g_cur, g_nxt = (ga, gb) if k % 2 == 0 else (gb, ga)
                    for m in range(NB):
                        out = Rq[m][:, :]
                        for kb in range(NB):
                            if k == 0 and kb == m:
                                lhsT = mtg[:, m * 128 : (m + 1) * 128]
                            else:
                                lhsT = mtb[
                                    :, kb * N + m * 128 : kb * N + (m + 1) * 128
                                ]
                            # start marks the WHOLE psum bank pending-zero:
                            # exactly once per bank lifetime
                            nc.tensor.matmul(
                                out,
                                lhsT,
                                g_cur[:, kb * W : (kb + 1) * W],
                                start=(k == 0 and kb == 0),
                                stop=(k == K - 2 and kb == NB - 1),
                                skip_group_check=True,
                            )
                    # x accumulation (PE, one 320-row matmul vs identity)
                    nc.tensor.matmul(
                        Z, idb, g_cur, start=(k == 0), stop=False,
                        skip_group_check=True,
                    )
                    for h in range(NB):
                        sl = slice(h * W, (h + 1) * W)
                        eng = nc.vector
                        eng.scalar_tensor_tensor(
                            g_nxt[:, sl],
                            g_cur[:, sl],
                            BETA,
                            Rq[h][:, :],
                            op0=OP.mult,
                            op1=OP.add,
                        )
                    if k == S_EARLY:
                        # approximate S from the partial solution (U columns)
                        nc.scalar.activation(
                            ube.rearrange("p (b e) -> p b e", e=E),
                            z3[:, :, BS:W],
                            AF.Copy,
                        )
                        se_ps = pns.tile([E, E], F32, tag="ns")
                        for m in range(NB):
                            nc.tensor.matmul(
                                se_ps,
                                a0tb[:, m * E : (m + 1) * E],
                                ube[:, m * E : (m + 1) * E],
                                start=(m == 0),
                                stop=(m == NB - 1),
                            )
                        nc.vector.scalar_tensor_tensor(
                            xb, se_ps, B_NS, aeye, op0=OP.mult, op1=OP.add
                        )
                        nc.scalar.activation(seb, se_ps, AF.Copy)
                    if k == S_EARLY + 1:
                        ns_emit(0)

                # final increment accumulation
                g_fin = (ga, gb)[(K - 1) % 2]
                nc.tensor.matmul(
                    Z, idb, g_fin, start=False, stop=True, skip_group_check=True
                )
                # zb copies BEFORE the remaining NS emission so the ACT queue
                # is not head-blocked by the NS chain
                nc.scalar.activation(zb3[:, :, BS:W], z3[:, :, BS:W], AF.Copy)
                nc.vector.tensor_copy(zb3[:, :, 0:BS], z3[:, :, 0:BS])
                for j in range(1, J_E):
                    ns_emit(j)

            # ---- tail ----
            with tc.tile_pool(name="ptail", bufs=1, space="PSUM") as ptail:
                sd_ps = ptail.tile([E, E + BS], F32, tag="sd")
                s_ps = sd_ps[:, 0:E]
                d_ps = sd_ps[:, E : E + BS]
                for m in range(NB):
                    nc.tensor.matmul(
                        s_ps,
                        a0tb[:, m * E : (m + 1) * E],
                        zb3[:, m, BS:W],
                        start=(m == 0),
                        stop=(m == NB - 1),
                    )
                nc.vector.tensor_copy(s_sb, s_ps)
                for m in range(NB):
                    nc.tensor.matmul(
                        d_ps,
                        a0tb[:, m * E : (m + 1) * E],
                        zb3[:, m, 0:BS],
                        start=(m == 0),
                        stop=(m == NB - 1),
                    )
                nc.vector.tensor_tensor(dsb, d_ps, cpos, op=OP.add)

                # U^T blocks for the final correction matmuls
                ut_ps = ptail.tile([E, NB * 128], BF16, tag="ut")
                for m in range(NB):
                    nc.tensor.transpose(
                        ut_ps[:, m * 128 : (m + 1) * 128], zb3[:, m, BS:W], idb
                    )
                nc.vector.tensor_copy(utb, ut_ps)

                # fused fp32 polish + W: W = X(2I - S X)D as 16-col matmuls:
                #   t1 = X D;  t2 = S t1;  t3 = X t2;  -W = t3 - 2 t1
                nc.vector.tensor_copy(xns, xb)
                t1_ps = ptail.tile([E, BS], F32, tag="nsf")
                nc.tensor.matmul(t1_ps, xns, dsb, start=True, stop=True)
                t1c = state.tile([E, BS], F32, tag="t1c")
                nc.vector.tensor_copy(t1c, t1_ps)
                t2_ps = ptail.tile([E, BS], F32, tag="nsf")
                nc.tensor.matmul(t2_ps, s_sb, t1c, start=True, stop=True)
                t2c = state.tile([E, BS], F32, tag="t2c")
                nc.vector.tensor_copy(t2c, t2_ps)
                t3_ps = ptail.tile([E, BS], F32, tag="nsf")
                nc.tensor.matmul(t3_ps, xns, t2c, start=True, stop=True)
                nc.vector.scalar_tensor_tensor(
                    wb, t1c, -2.0, t3_ps, op0=OP.mult, op1=OP.add
                )
                if dbg:
                    nc.sync.dma_start(s_dbg[:, :], s_sb)
                    nc.sync.dma_start(x_dbg[:, :], xns)
                    nc.gpsimd.dma_start(w_dbg[:, :], wb)

                for m in range(NB):
                    nc.tensor.matmul(
                        z3[:, m, 0:BS],
                        utb[:, m * 128 : (m + 1) * 128],
                        wb,
                        start=False,
                        stop=True,
                        skip_group_check=True,
                    )
                nc.vector.tensor_copy(
                    ysb.rearrange("p (b j) -> p b j", j=BS), z3[:, :, 0:BS]
                )
                if dbg:
                    zc = state.tile([128, TW], F32, tag="zc")
                    nc.vector.tensor_copy(zc, Z)
                    nc.sync.dma_start(z_dbg[:, :], zc)
                nc.sync.dma_start(y_d[:, :], ysb)

    nc.compile()
    return nc


def _bf16(a):
    return np.asarray(a, dtype=np.float32).astype(BF)


def _prep_blobs(x, parms, M, A0, c):
    """Host-side layout/dtype marshalling (no input-data math)."""
    Mt = M + 2.0 * np.eye(N, dtype=np.float32)
    Mh = (-ALPHA * Mt).astype(np.float32)
    b16 = np.zeros((128, C16), dtype=BF)
    for m in range(NB):
        b16[:, C_A0 + m * E : C_A0 + (m + 1) * E] = _bf16(
            A0[:, m * 128 : (m + 1) * 128].T
        )
    b16[:, C_ID : C_ID + 128] = _bf16(np.eye(128, dtype=np.float32))
    for m in range(NB):
        blk = Mh[m * 128 : (m + 1) * 128, m * 128 : (m + 1) * 128] + np.eye(
            128, dtype=np.float32
        )
        b16[:, C_MG + m * 128 : C_MG + (m + 1) * 128] = _bf16(blk)
    for kb in range(NB):
        b16[:, C_MT + kb * N : C_MT + (kb + 1) * N] = _bf16(
            Mh[kb * 128 : (kb + 1) * 128, :]
        )

    base = np.zeros((128, C32), dtype=np.float32)
    base[0:E, F_AE : F_AE + E] = A_NS * np.eye(E, dtype=np.float32)
    base[0:E, F_CP : F_CP + BS] = np.repeat(c.reshape(E, 1), BS, axis=1)
    b32s = []
    for i in range(NCORES):
        b32 = base.copy()
        xs = x[i * BS : (i + 1) * BS]
        ps = parms[i * BS : (i + 1) * BS]
        for m in range(NB):
            b32[:, F_XT + m * BS : F_XT + (m + 1) * BS] = xs[
                :, m * 128 : (m + 1) * 128
            ].T
            b32[:, F_PT + m * BS : F_PT + (m + 1) * BS] = ps[
                :, m * 128 : (m + 1) * 128
            ].T
        b32s.append(np.ascontiguousarray(b32))
    return np.ascontiguousarray(b16), b32s


def _ensure_axon_ntff_hook():
    """Provide antenv.axon_hooks if the image lacks it (profiling only)."""
    try:
        import antenv.axon_hooks  # noqa: F401

        return
    except ImportError:
        pass
    import contextlib
    import ctypes
    import types

    hook = None
    so_path = "/opt/axon/libaxon_pjrt.so"
    if os.path.exists(so_path):
        lib = ctypes.CDLL(so_path)
        if hasattr(lib, "axon_start_nrt_profile"):
            lib.axon_start_nrt_profile.argtypes = [
                ctypes.POINTER(ctypes.c_int64),
                ctypes.c_size_t,
            ]
            lib.axon_start_nrt_profile.restype = ctypes.c_int64
            lib.axon_stop_nrt_profile.argtypes = [ctypes.c_char_p]
            lib.axon_stop_nrt_profile.restype = ctypes.c_int64

            @contextlib.contextmanager
            def _hook(output_dir, device_ids):
                import jax

                jax.devices()
                if device_ids:
                    ids = (ctypes.c_int64 * len(device_ids))(*device_ids)
                    rc = lib.axon_start_nrt_profile(ids, len(device_ids))
                else:
                    rc = lib.axon_start_nrt_profile(None, 0)
                if rc != 0:
                    raise RuntimeError(f"axon_start_nrt_profile rc={rc}")
                try:
                    yield
                finally:
                    n = lib.axon_stop_nrt_profile(str(output_dir).encode())
                    print(f"ntff profile: {n} file(s) -> {output_dir}")

            hook = _hook

    mod = types.ModuleType("antenv.axon_hooks")
    mod.get_axon_ntff_profile_hook = lambda: hook
    mod.set_axon_ntff_profile_hook = lambda h: None
    sys.modules["antenv.axon_hooks"] = mod


_NC_CACHE = {}


def kernel(x, parms, M, A0, B0=None, c=None, **_unused):
    x = np.ascontiguousarray(x, dtype=np.float32)
    parms = np.ascontiguousarray(parms, dtype=np.float32)
    M = np.ascontiguousarray(M, dtype=np.float32)
    A0 = np.ascontiguousarray(A0, dtype=np.float32)
    c = np.ascontiguousarray(c, dtype=np.float32).reshape(E)

    dbg = bool(int(os.environ.get("KERNEL_DEBUG", "0")))
    if dbg not in _NC_CACHE:
        _NC_CACHE[dbg] = build_bass(dbg)
    nc = _NC_CACHE[dbg]

    b16, b32s = _prep_blobs(x, parms, M, A0, c)
    in_maps = [{"b16": b16, "b32": b32s[i]} for i in range(NCORES)]

    trace = bool(int(os.environ.get("KERNEL_TRACE", "0")))
    if trace:
        _ensure_axon_ntff_hook()
    res = run_bass_kernel_spmd(
        nc, in_maps, core_ids=list(range(NCORES)), trace=trace
    )
    LAST_RUN["exec_time_ns"] = res.exec_time_ns
    LAST_RUN["mean_exec_time_ns"] = res.mean_exec_time_ns
    LAST_RUN["trace"] = res.instructions_and_trace
    LAST_RUN["profile_json"] = res.profile_json
    LAST_RUN["debug"] = {
        k: v for k, v in res.results[0].items() if k.startswith("dbg_")
    }
    out = np.empty((B, N), dtype=np.float32)
    for i, r in enumerate(res.results):
        yc = np.asarray(r["y"], dtype=np.float32)  # [128, m*BS+j]
        for m in range(NB):
            out[i * BS : (i + 1) * BS, m * 128 : (m + 1) * 128] = yc[
                :, m * BS : (m + 1) * BS
            ].T
    return out
